# revision 43
# baseline (speedup 1.0000x reference)
"""Multi-head attention (B=2, S=2048, H=16, DH=64, D=1024) on 8 TRN2 NeuronCores.

Sharding: batch x head-group. Core c handles batch b = c//4, head group
hg = c%4 (4 heads = 256 hidden columns). Each core computes its head group's
attention and a partial (row-sliced) output projection; the host sums the 4
partials per batch and adds the bias terms.

Device-side dataflow (per core):
  - hsT [D, S] arrives pre-transposed bf16; Wq/Wk/Wv bf16, Wo fp32r.
  - qT/kT = W-contract matmuls (bf16, K=1024 over 8 chunks); the per-
    partition bias add runs on DVE (tensor_scalar_add), not ScalarE --
    ScalarE is the kernel's bottleneck engine (see below).
  - v = hs @ Wv in natural [S, 256] layout, written fp8e4m3 into
    v_sb [128, skc, head, 80]: cols 0:64 = v, col 64 = ones (softmax
    denominator trick), 65:80 pad for DoubleRow's 16B k-tile stride.
  - Per head pair: two K=64 scoresT [sk, sq] bf16 matmuls in disjoint PE
    row groups (partition bases 0/64) overlap in the array; one ScalarE
    exp(scale*x) covers both heads -> fp8e4m3 into a [128, 2, 1024]
    chunk-pair tile (no max-subtraction: scores bounded ~+-5).
    The ScalarE exp stream (128 x ~1us per rep) is the throughput limit;
    everything else is scheduled around keeping it saturated.
  - PV: fp8 DoubleRow matmuls, K=256 (two sk chunks per instruction,
    2 MACs/cell/cycle): lhsT = v pair-slice [128, 2, 65], rhs = exp pair
    [128, 2, 512] -> ctxT_aug [65, sq] accumulated over 8 chunk pairs.
    Row 64 accumulates the denominators l.
  - l -> 1/l (DVE reciprocal), broadcast across 64 partitions via a K=1
    matmul with a ones column, ctxT = ctxT_unnorm * bcast(1/l) (fp32r).
  - outT_partial [D, S] = Wo-contract (fp32r), written bf16 (halves the
    output DMA, which contends hard on HW).
Scheduling: only kT(0,md0)+qT(0,md0) run before the attention units, so
the exp stream starts as soon as ~2MB of input lands. All other
projections, the v chunks, epilogues and out-projections are deadline-
ordered "drip" items consumed one per sk-chunk inside the units, sized
<=4 matmuls so they never stall the scores->exp chain; their psum scratch
draws from the ctx pool, never from the scores double-buffer. For reps>1
(timing NEFFs), rep r+1's inputs are DMA-prefetched during rep r's last
unit and all per-rep SBUF tensors are double-buffered, so consecutive reps
pipeline. Host: out[b] = sum_hg(outT_partial).T + (bo + bv @ Wo) (bv folds
through the probs@V linearity: probs rows sum to 1 after normalization).
"""

import numpy as np

H = 16
DH = 64
D = 1024
B = 2
S = 2048
HG = 4            # heads per core
DG = HG * DH      # 256 hidden cols per core
SCALE = DH ** -0.5
N_CORES = 8

_cached_nc = None


def _build_nc(reps=1):
    import concourse.bass as bass  # noqa: F401
    from concourse import bacc
    import concourse.mybir as mybir
    import concourse.tile as tile

    F32 = mybir.dt.float32
    F32R = mybir.dt.float32r
    BF16 = mybir.dt.bfloat16
    FP8 = mybir.dt.float8e4
    DR = mybir.MatmulPerfMode.DoubleRow
    AFT = mybir.ActivationFunctionType

    nc = bacc.Bacc("TRN2", target_bir_lowering=False)

    hsT = nc.dram_tensor("hsT", [D, S], BF16, kind="ExternalInput")
    wq = nc.dram_tensor("wq", [D, DG], BF16, kind="ExternalInput")
    wk = nc.dram_tensor("wk", [D, DG], BF16, kind="ExternalInput")
    wv = nc.dram_tensor("wv", [D, DG], BF16, kind="ExternalInput")
    wo = nc.dram_tensor("wo", [DG, D], F32R, kind="ExternalInput")
    bq = nc.dram_tensor("bq", [2, 128], mybir.dt.float32, kind="ExternalInput")
    bk = nc.dram_tensor("bk", [2, 128], mybir.dt.float32, kind="ExternalInput")
    outT = nc.dram_tensor("outT", [D, S], BF16, kind="ExternalOutput")

    KC = D // 128     # 8 contraction chunks for projections
    SQC = S // 512    # 4 sq chunks of 512
    SKC = S // 128    # 16 sk chunks of 128

    with tile.TileContext(nc) as tc:
        with tc.tile_pool(name="cst", bufs=1) as cst, \
             tc.tile_pool(name="big", bufs=2) as big, \
             tc.tile_pool(name="wop", bufs=1) as wop, \
             tc.tile_pool(name="expp", bufs=4) as expp, \
             tc.tile_pool(name="ep", bufs=3) as ep, \
             tc.tile_pool(name="ost", bufs=10) as ost, \
             tc.tile_pool(name="qkp", bufs=2) as qkp, \
             tc.tile_pool(name="pbig", bufs=2, space="PSUM") as pbig, \
             tc.tile_pool(name="pctx", bufs=4, space="PSUM") as pctx:

            # ---- constants, emitted once (reps share them) ----
            ones_f = cst.tile([65, 64], F32)
            ones_r = cst.tile([65, 64], F32R)
            vones_f = cst.tile([128, SKC, HG, 1], F32)
            nc.vector.memset(ones_f, 1.0)
            nc.vector.tensor_copy(ones_r, ones_f)
            nc.vector.memset(vones_f, 1.0)

            # ---- PE warmup, once: dummy matmuls on locally-produced ones
            # data run during the initial input-DMA wait, so the HAM clock
            # gate is already at 8/8 (2.4 GHz) when the real projections
            # start. Later reps inherit a warm PE.
            warm = pbig.tile([128, 512], mybir.dt.float32, tag="st", name="warm")
            for wi in range(36):
                nc.tensor.matmul(
                    warm[0:64, 0:64], ones_r[0:64, 0:64], ones_r[0:64, 0:64],
                    start=(wi == 0), stop=(wi == 35),
                )

            def alloc_tiles():
                # per-rep SBUF tensors; bufs=2 on `big` pipelines rep r+1's
                # input DMAs and projections under rep r's tail. wo is read
                # only by the tail out-projections, so it stays
                # single-buffered to fit SBUF.
                return dict(
                    hsT_sb=big.tile([128, KC, S], BF16, name="hsT_sb"),
                    wq_sb=big.tile([128, KC, DG], BF16, name="wq_sb"),
                    wk_sb=big.tile([128, KC, DG], BF16, name="wk_sb"),
                    wv_sb=big.tile([128, KC, DG], BF16, name="wv_sb"),
                    wo_sb=wop.tile([128, 2, D], F32R, name="wo_sb"),
                    bq_sb=big.tile([128, 2], mybir.dt.float32, name="bq_sb"),
                    bk_sb=big.tile([128, 2], mybir.dt.float32, name="bk_sb"),
                    qT_sb=big.tile([128, 2, S], BF16, name="qT_sb"),
                    kT_sb=big.tile([128, 2, S], BF16, name="kT_sb"),
                    # v for PV DoubleRow: [sk_within_chunk, skc, head, 80]
                    # fp8; cols 0:64 = v, col 64 = ones (denominator), 65:80
                    # pad so the k-tile stride (HG*80 bytes) is 16B-aligned.
                    v_sb=big.tile([128, SKC, HG, 80], FP8, name="v_sb"),
                    ctxT_sb=big.tile([128, 2, S], F32R, name="ctxT_sb"),
                )

            def emit_dmas(T):
                # DMA order = first-use order: wk+hsT(sqc0) gate kT(0,md0),
                # wq gates qT(0,md0) (these two projections alone start the
                # exp stream); wv gates the inline v chunks; the rest streams
                # under the attention units. One DMA per tensor/sqc-block:
                # the ~0.6us fixed cost per DMA dominates the lead-in if the
                # loads are split per-kc.
                wk_r = wk[:, :].rearrange("(kc p) n -> p kc n", p=128)
                wq_r = wq[:, :].rearrange("(kc p) n -> p kc n", p=128)
                wv_r = wv[:, :].rearrange("(kc p) n -> p kc n", p=128)
                nc.sync.dma_start(out=T["bk_sb"], in_=bk[:, :].rearrange("md p -> p md"))
                nc.sync.dma_start(out=T["bq_sb"], in_=bq[:, :].rearrange("md p -> p md"))
                hsT_r = hsT[:, :].rearrange("(kc p) s -> p kc s", p=128)
                nc.sync.dma_start(out=T["wk_sb"], in_=wk_r)
                nc.sync.dma_start(out=T["hsT_sb"][:, :, 0:512], in_=hsT_r[:, :, 0:512])
                nc.sync.dma_start(out=T["wq_sb"], in_=wq_r)
                nc.sync.dma_start(out=T["wv_sb"], in_=wv_r)
                for sqc in range(1, SQC):
                    nc.sync.dma_start(
                        out=T["hsT_sb"][:, :, sqc * 512:(sqc + 1) * 512],
                        in_=hsT_r[:, :, sqc * 512:(sqc + 1) * 512],
                    )
                wo_r = wo[:, :].rearrange("(kc p) n -> p kc n", p=128)
                nc.sync.dma_start(out=T["wo_sb"], in_=wo_r)

            def emit_body(T, nextT):
                hsT_sb = T["hsT_sb"]
                wq_sb = T["wq_sb"]
                wk_sb = T["wk_sb"]
                wv_sb = T["wv_sb"]
                wo_sb = T["wo_sb"]
                bq_sb = T["bq_sb"]
                bk_sb = T["bk_sb"]
                qT_sb = T["qT_sb"]
                kT_sb = T["kT_sb"]
                v_sb = T["v_sb"]
                ctxT_sb = T["ctxT_sb"]

                # ---- v ones column (per rep: v_sb rotates) ----
                nc.vector.tensor_copy(v_sb[:, :, :, 64:65], vones_f)

                # ---- helper emitters ----
                def qk_proj_pieces(w_sb, b_sb, dst, sqc, md):
                    # one qT/kT projection block as 2 self-contained drip
                    # items of 4 K-chunk matmuls each (~850ns PE), so a
                    # dripped projection neither stalls the next scores
                    # matmuls for long nor holds a psum buffer across drip
                    # slots (which would single-buffer the scores psum and
                    # stall the exp stream). Halves meet through a bf16 SBUF
                    # partial; bias-adds run on DVE (ScalarE is exp-bound).
                    ssl = slice(sqc * 512, (sqc + 1) * 512)
                    msl = slice(md * 128, (md + 1) * 128)
                    st = {}

                    def piece_a():
                        ps = pctx.tile([128, 512], mybir.dt.float32, tag="ctx", name="ps_qk")
                        for kc in range(4):
                            nc.tensor.matmul(
                                ps, w_sb[:, kc, msl], hsT_sb[:, kc, ssl],
                                start=(kc == 0), stop=(kc == 3),
                            )
                        st["half"] = qkp.tile([128, 512], BF16, name="qk_part")
                        nc.vector.tensor_scalar_add(st["half"], ps, b_sb[:, md:md + 1])

                    def piece_b():
                        ps = pctx.tile([128, 512], mybir.dt.float32, tag="ctx", name="ps_qk")
                        for kc in range(4, KC):
                            nc.tensor.matmul(
                                ps, w_sb[:, kc, msl], hsT_sb[:, kc, ssl],
                                start=(kc == 4), stop=(kc == KC - 1),
                            )
                        nc.vector.tensor_add(dst[:, md, ssl], ps, st["half"])
                    return [piece_a, piece_b]

                def qk_proj(w_sb, b_sb, dst, sqc, md):
                    for p in qk_proj_pieces(w_sb, b_sb, dst, sqc, md):
                        p()

                def v_proj(skc):
                    # psum from the ctx pool: during unit 0 (the only v_proj
                    # window) it has free slots, while taking a pbig buffer
                    # would single-buffer the scores psum under the exp stream.
                    ksl = slice(skc * 128, (skc + 1) * 128)
                    psv = pctx.tile([128, DG], mybir.dt.float32, tag="ctx", name="psv")
                    for kc in range(KC):
                        nc.tensor.matmul(
                            psv, hsT_sb[:, kc, ksl], wv_sb[:, kc, :],
                            start=(kc == 0), stop=(kc == KC - 1),
                        )
                    nc.vector.tensor_copy(
                        v_sb[:, skc, :, 0:64],
                        psv.rearrange("p (h d) -> p h d", h=HG),
                    )

                def epilogue(h, ctx, sqc):
                    # normalize ctxT_unnorm (rows 0:64) by l (row 64), write ctxT
                    poff = (h % 2) * 64
                    cpart = h // 2
                    ssl = slice(sqc * 512, (sqc + 1) * 512)
                    invl_r = ep.tile([65, 512], F32R, tag="invr", name="invl_r")
                    with nc.allow_low_precision(reason="1/l rounded to fp32r feeds the fp32r broadcast matmul"):
                        nc.vector.reciprocal(invl_r[64:65, :], ctx[64:65, :])
                    psb = pctx.tile([64, 512], mybir.dt.float32, tag="ctx", name="psb")
                    nc.tensor.matmul(
                        psb, ones_r[64:65, 0:64], invl_r[64:65, :],
                        start=True, stop=True,
                    )
                    bc = ep.tile([64, 512], mybir.dt.float32, tag="bc", name="bc")
                    nc.vector.tensor_copy(bc, psb)
                    nc.vector.tensor_mul(
                        ctxT_sb[poff:poff + 64, cpart, ssl],
                        ctx[0:64, :], bc,
                    )

                out_copy_toggle = [0]
                in_tail = [False]

                def out_proj(mo, sqc):
                    osl = slice(mo * 128, (mo + 1) * 128)
                    ssl = slice(sqc * 512, (sqc + 1) * 512)
                    pso = pctx.tile([128, 512], mybir.dt.float32, tag="ctx", name="pso")
                    for kc2 in range(2):
                        nc.tensor.matmul(
                            pso, wo_sb[:, kc2, osl], ctxT_sb[:, kc2, ssl],
                            start=(kc2 == 0), stop=(kc2 == 1),
                        )
                    ot = ost.tile([128, 512], BF16, name="ot")
                    if in_tail[0] and out_copy_toggle[0] % 2 == 0:
                        nc.scalar.copy(out=ot, in_=pso)
                    else:
                        nc.vector.tensor_copy(ot, pso)
                    out_copy_toggle[0] += 1
                    nc.sync.dma_start(out=outT[osl, ssl], in_=ot)

                # ---- pre-attention: ONLY kT(0,md0) + qT(0,md0) ----
                # These two projections alone gate unit (0,0)'s first scores,
                # so the exp stream (the ScalarE bottleneck) starts as soon as
                # wk+wq+hsT(sqc0) land. Every other projection and all v
                # chunks are deadline-ordered into the units' drip stream.
                qk_proj(wk_sb, bk_sb, kT_sb, 0, 0)
                qk_proj(wq_sb, bq_sb, qT_sb, 0, 0)

                # deferred work, dripped into later attention units' loops.
                # prio_q (qT projections + ctx-psum-releasing epilogues) drains
                # ahead of slack_q and every iteration, so unit u's ctx slots free
                # within unit u+1 and qT(sqc) is ready one sqc ahead.
                from collections import deque
                prio_q = deque()
                slack_q = deque()

                def drip(slack_ok=True):
                    if prio_q:
                        prio_q.popleft()()
                    elif slack_ok and slack_q:
                        slack_q.popleft()()

                # ---- attention units: (sqc, head-pair), software-pipelined ----
                # Each pair's two K=64 scoresT matmuls go to disjoint PE row
                # groups (partitions 0:64 / 64:128) and the two banks of one
                # [128, 1024] psum tile, so the PE overlaps them and a single exp
                # covers both heads.
                units = [(sqc, pair) for sqc in range(SQC) for pair in range(2)]
                for ui, (sqc, pair) in enumerate(units):
                    ssl = slice(sqc * 512, (sqc + 1) * 512)
                    ctxs = [None, None]
                    # deadline-ordered projection drips. kT(s, md) is read by
                    # every unit with pair==md at its skc=4s; qT(s, md) gates
                    # unit (s, md)'s start.
                    if ui == 0:
                        for k_sqc in (1, 2, 3):
                            prio_q.extend(qk_proj_pieces(wk_sb, bk_sb, kT_sb, k_sqc, 0))
                        prio_q.extend(qk_proj_pieces(wk_sb, bk_sb, kT_sb, 0, 1))
                        prio_q.extend(qk_proj_pieces(wq_sb, bq_sb, qT_sb, 0, 1))
                        prio_q.extend(qk_proj_pieces(wk_sb, bk_sb, kT_sb, 1, 1))
                    if ui == 1:
                        for k_sqc in (2, 3):
                            prio_q.extend(qk_proj_pieces(wk_sb, bk_sb, kT_sb, k_sqc, 1))
                        for md in range(2):
                            prio_q.extend(qk_proj_pieces(wq_sb, bq_sb, qT_sb, 1, md))
                    if ui in (2, 3):
                        for md in range(2):
                            prio_q.extend(qk_proj_pieces(wq_sb, bq_sb, qT_sb, ui, md))
                    # exp is written fp8 into [128, 2, 1024] chunk-pair tiles;
                    # PV runs as fp8 DoubleRow matmuls contracting 2 sk-chunks
                    # (K=256) per instruction at 2 MACs/cell/cycle.
                    prev = None
                    cur = None
                    for skc in range(SKC):
                        ksl = slice(skc * 128, (skc + 1) * 128)
                        t = skc % 2
                        if t == 0:
                            cur = expp.tile([128, 2, 1024], FP8, name="expT2")
                        sT = pbig.tile([128, 1024], mybir.dt.float32, tag="st", name="sT")
                        for hh in range(2):
                            nc.tensor.matmul(
                                sT[:, hh * 512:(hh + 1) * 512],
                                kT_sb[hh * 64:(hh + 1) * 64, pair, ksl],
                                qT_sb[hh * 64:(hh + 1) * 64, pair, ssl],
                                start=True, stop=True,
                            )
                        nc.scalar.activation(cur[:, t, :], sT, AFT.Exp, scale=SCALE)
                        if skc == 3:
                            # ctx psum allocated lazily, after the previous
                            # unit's first epilogue has dripped (skc==2): an
                            # eager alloc would fill the pool and deadlock the
                            # epilogue's own psum allocation.
                            ctxs[0] = pctx.tile([65, 512], mybir.dt.float32, tag="ctx", name="ctx0")
                            ctxs[1] = pctx.tile([65, 512], mybir.dt.float32, tag="ctx", name="ctx1")
                        if t == 1:
                            if prev is not None:
                                pexp, ppc = prev
                                for hh in range(2):
                                    nc.tensor.matmul(
                                        ctxs[hh],
                                        v_sb[:, 2 * ppc:2 * ppc + 2, pair * 2 + hh, 0:65],
                                        pexp[:, :, hh * 512:(hh + 1) * 512],
                                        start=(ppc == 0), stop=False,
                                        perf_mode=DR,
                                    )
                            prev = (cur, skc // 2)
                        if ui == 0:
                            v_proj(skc)  # stream all 16 v chunks under unit 0
                        if ui == 7 and skc == 0 and nextT is not None:
                            # prefetch the next rep's inputs now: they enter
                            # the sync queue ahead of this rep's tail output
                            # DMAs, and the DMA engines are idle here, so the
                            # next rep's pre-attention starts at the boundary
                            # with data already resident.
                            emit_dmas(nextT)
                        if skc >= 2:
                            drip()
                    pexp, ppc = prev
                    for hh in range(2):
                        nc.tensor.matmul(
                            ctxs[hh],
                            v_sb[:, 2 * ppc:2 * ppc + 2, pair * 2 + hh, 0:65],
                            pexp[:, :, hh * 512:(hh + 1) * 512],
                            start=(ppc == 0), stop=True,
                            perf_mode=DR,
                        )
                    # defer this unit's epilogues into the next unit's loop
                    for hh in range(2):
                        prio_q.append(
                            lambda h=pair * 2 + hh, ctx=ctxs[hh], sqc=sqc: epilogue(h, ctx, sqc))
                    if pair == 1:
                        # both pairs of this sqc done (after epilogues): queue out-proj
                        for mo in range(8):
                            slack_q.append(lambda mo=mo, sqc=sqc: out_proj(mo, sqc))

                # drain remaining deferred work
                in_tail[0] = True
                while prio_q or slack_q:
                    drip(slack_ok=True)

            T = alloc_tiles()
            emit_dmas(T)
            for _rep in range(reps):
                nextT = alloc_tiles() if _rep + 1 < reps else None
                emit_body(T, nextT)
                T = nextT

    nc.compile()
    return nc


def _get_nc(reps=1):
    global _cached_nc
    if reps != 1:
        return _build_nc(reps)
    if _cached_nc is None:
        _cached_nc = _build_nc()
    return _cached_nc


def kernel(hidden_states, Wq, bq, Wk, bk, Wv, bv, Wo, bo, _want_trace=False):
    from concourse.bass_utils import run_bass_kernel_spmd

    hidden_states = np.asarray(hidden_states, dtype=np.float32)
    Wq = np.asarray(Wq, dtype=np.float32)
    Wk = np.asarray(Wk, dtype=np.float32)
    Wv = np.asarray(Wv, dtype=np.float32)
    Wo = np.asarray(Wo, dtype=np.float32)
    bq = np.asarray(bq, dtype=np.float32)
    bk = np.asarray(bk, dtype=np.float32)
    bv = np.asarray(bv, dtype=np.float32)
    bo = np.asarray(bo, dtype=np.float32)

    nc = _get_nc()

    import ml_dtypes
    bf16 = ml_dtypes.bfloat16
    hsTs = [np.ascontiguousarray(hidden_states[b].T).astype(bf16) for b in range(B)]
    in_maps = []
    for c in range(N_CORES):
        b, hg = divmod(c, HG)
        sl = slice(hg * DG, (hg + 1) * DG)
        in_maps.append({
            "hsT": hsTs[b],
            "wq": np.ascontiguousarray(Wq[:, sl]).astype(bf16),
            "wk": np.ascontiguousarray(Wk[:, sl]).astype(bf16),
            "wv": np.ascontiguousarray(Wv[:, sl]).astype(bf16),
            "wo": np.ascontiguousarray(Wo[sl, :]),
            "bq": np.ascontiguousarray(bq[sl].reshape(2, 128)),
            "bk": np.ascontiguousarray(bk[sl].reshape(2, 128)),
        })

    try:
        res = run_bass_kernel_spmd(
            nc, in_maps, core_ids=list(range(N_CORES)), trace=_want_trace,
        )
    except ModuleNotFoundError:
        res = run_bass_kernel_spmd(
            nc, in_maps, core_ids=list(range(N_CORES)), trace=False,
        )

    bias_full = bo + bv @ Wo  # [D]
    out = np.empty((B, S, D), dtype=np.float32)
    for b in range(B):
        acc = res.results[HG * b]["outT"].astype(np.float64)
        for g in range(1, HG):
            acc = acc + res.results[HG * b + g]["outT"]
        out[b] = acc.T + bias_full

    if _want_trace:
        return out, res
    return out



# revision 56
# speedup vs baseline: 1.1543x; 1.1543x over previous
"""Multi-head attention (B=2, S=2048, H=16, DH=64, D=1024) on 8 TRN2 NeuronCores.

Sharding: batch x head-group. Core c handles batch b = c//4, head group
hg = c%4 (4 heads = 256 hidden columns). Each core computes its head group's
attention and a partial (row-sliced) output projection; the host sums the 4
partials per batch and adds the bias terms.

Device-side dataflow (per core):
  - hsT [D, S] arrives pre-transposed bf16; Wq/Wk/Wv bf16, Wo fp32r.
  - qT/kT = W-contract matmuls (bf16, K=1024 over 8 chunks); the per-
    partition bias add runs on DVE (tensor_scalar_add), not ScalarE --
    ScalarE is the kernel's bottleneck engine (see below).
  - v = hs @ Wv in natural [S, 256] layout, written fp8e4m3 into
    v_sb [128, skc, head, 80]: cols 0:64 = v, col 64 = ones (softmax
    denominator trick), 65:80 pad for DoubleRow's 16B k-tile stride.
  - Per head pair: two K=64 scoresT [sk, sq] bf16 matmuls in disjoint PE
    row groups (partition bases 0/64) overlap in the array; one ScalarE
    exp(scale*x) covers both heads -> fp8e4m3 into a [128, 2, 1024]
    chunk-pair tile (no max-subtraction: scores bounded ~+-5).
    The ScalarE exp stream (128 x ~1us per rep) is the throughput limit;
    everything else is scheduled around keeping it saturated.
  - PV: fp8 DoubleRow matmuls, K=256 (two sk chunks per instruction,
    2 MACs/cell/cycle): lhsT = v pair-slice [128, 2, 65], rhs = exp pair
    [128, 2, 512] -> ctxT_aug [65, sq] accumulated over 8 chunk pairs.
    Row 64 accumulates the denominators l.
  - l -> 1/l (DVE reciprocal), broadcast across 64 partitions via a K=1
    matmul with a ones column, ctxT = ctxT_unnorm * bcast(1/l) (fp32r).
  - outT_partial [D, S] = Wo-contract (fp32r), written bf16 (halves the
    output DMA, which contends hard on HW).
Scheduling: only kT(0,md0)+qT(0,md0) run before the attention units, so
the exp stream starts as soon as ~2MB of input lands. All other
projections, the v chunks, epilogues and out-projections are deadline-
ordered "drip" items consumed one per sk-chunk inside the units, sized
<=4 matmuls so they never stall the scores->exp chain; their psum scratch
draws from the ctx pool, never from the scores double-buffer. For reps>1
(timing NEFFs), rep r+1's inputs are DMA-prefetched during rep r's last
unit and all per-rep SBUF tensors are double-buffered, so consecutive reps
pipeline. Host: out[b] = sum_hg(outT_partial).T + (bo + bv @ Wo) (bv folds
through the probs@V linearity: probs rows sum to 1 after normalization).
"""

import numpy as np

H = 16
DH = 64
D = 1024
B = 2
S = 2048
HG = 4            # heads per core
DG = HG * DH      # 256 hidden cols per core
SCALE = DH ** -0.5
N_CORES = 8

_cached_nc = None


def _build_nc(reps=1, carry=True):
    import concourse.bass as bass  # noqa: F401
    from concourse import bacc
    import concourse.mybir as mybir
    import concourse.tile as tile

    F32 = mybir.dt.float32
    F32R = mybir.dt.float32r
    BF16 = mybir.dt.bfloat16
    FP8 = mybir.dt.float8e4
    DR = mybir.MatmulPerfMode.DoubleRow
    AFT = mybir.ActivationFunctionType

    nc = bacc.Bacc("TRN2", target_bir_lowering=False)

    hsT = nc.dram_tensor("hsT", [D, S], BF16, kind="ExternalInput")
    wq = nc.dram_tensor("wq", [D, DG], BF16, kind="ExternalInput")
    wk = nc.dram_tensor("wk", [D, DG], BF16, kind="ExternalInput")
    wv = nc.dram_tensor("wv", [D, DG], BF16, kind="ExternalInput")
    wo = nc.dram_tensor("wo", [DG, D], F32R, kind="ExternalInput")
    bq = nc.dram_tensor("bq", [2, 128], mybir.dt.float32, kind="ExternalInput")
    bk = nc.dram_tensor("bk", [2, 128], mybir.dt.float32, kind="ExternalInput")
    outT = nc.dram_tensor("outT", [D, S], BF16, kind="ExternalOutput")

    KC = D // 128     # 8 contraction chunks for projections
    SQC = S // 512    # 4 sq chunks of 512
    SKC = S // 128    # 16 sk chunks of 128

    with tile.TileContext(nc) as tc:
        with tc.tile_pool(name="cst", bufs=1) as cst, \
             tc.tile_pool(name="big", bufs=2) as big, \
             tc.tile_pool(name="wop", bufs=1) as wop, \
             tc.tile_pool(name="expp", bufs=4) as expp, \
             tc.tile_pool(name="ep", bufs=3) as ep, \
             tc.tile_pool(name="ost", bufs=10) as ost, \
             tc.tile_pool(name="qkp", bufs=3) as qkp, \
             tc.tile_pool(name="pbig", bufs=2, space="PSUM") as pbig, \
             tc.tile_pool(name="pctx", bufs=4, space="PSUM") as pctx:

            # ---- constants, emitted once (reps share them) ----
            ones_f = cst.tile([65, 64], F32)
            ones_r = cst.tile([65, 64], F32R)
            vones_f = cst.tile([128, SKC, HG, 1], F32)
            nc.vector.memset(ones_f, 1.0)
            nc.vector.tensor_copy(ones_r, ones_f)
            nc.vector.memset(vones_f, 1.0)

            # ---- PE warmup, once: dummy matmuls on locally-produced ones
            # data run during the initial input-DMA wait, so the HAM clock
            # gate is already at 8/8 (2.4 GHz) when the real projections
            # start. Later reps inherit a warm PE.
            warm = pbig.tile([128, 512], mybir.dt.float32, tag="st", name="warm")
            for wi in range(36):
                nc.tensor.matmul(
                    warm[0:64, 0:64], ones_r[0:64, 0:64], ones_r[0:64, 0:64],
                    start=(wi == 0), stop=(wi == 35),
                )

            def alloc_tiles():
                # per-rep SBUF tensors; bufs=2 on `big` pipelines rep r+1's
                # input DMAs and projections under rep r's tail. wo is read
                # only by the tail out-projections, so it stays
                # single-buffered to fit SBUF.
                return dict(
                    hsT_sb=big.tile([128, KC, S], BF16, name="hsT_sb"),
                    wq_sb=big.tile([128, KC, DG], BF16, name="wq_sb"),
                    wk_sb=big.tile([128, KC, DG], BF16, name="wk_sb"),
                    wv_sb=big.tile([128, KC, DG], BF16, name="wv_sb"),
                    wo_sb=wop.tile([128, 2, D], F32R, name="wo_sb"),
                    bq_sb=big.tile([128, 2], mybir.dt.float32, name="bq_sb"),
                    bk_sb=big.tile([128, 2], mybir.dt.float32, name="bk_sb"),
                    qT_sb=big.tile([128, 2, S], BF16, name="qT_sb"),
                    kT_sb=big.tile([128, 2, S], BF16, name="kT_sb"),
                    # v for PV DoubleRow: [sk_within_chunk, skc, head, 80]
                    # fp8; cols 0:64 = v, col 64 = ones (denominator), 65:80
                    # pad so the k-tile stride (HG*80 bytes) is 16B-aligned.
                    v_sb=big.tile([128, SKC, HG, 80], FP8, name="v_sb"),
                    ctxT_sb=big.tile([128, 2, S], F32R, name="ctxT_sb"),
                )

            def emit_dmas(T):
                # DMA order = first-use order: wk+hsT(sqc0) gate kT(0,md0),
                # wq gates qT(0,md0) (these two projections alone start the
                # exp stream); wv gates the inline v chunks; the rest streams
                # under the attention units. One DMA per tensor/sqc-block:
                # the ~0.6us fixed cost per DMA dominates the lead-in if the
                # loads are split per-kc.
                wk_r = wk[:, :].rearrange("(kc p) n -> p kc n", p=128)
                wq_r = wq[:, :].rearrange("(kc p) n -> p kc n", p=128)
                wv_r = wv[:, :].rearrange("(kc p) n -> p kc n", p=128)
                nc.sync.dma_start(out=T["bk_sb"], in_=bk[:, :].rearrange("md p -> p md"))
                nc.sync.dma_start(out=T["bq_sb"], in_=bq[:, :].rearrange("md p -> p md"))
                hsT_r = hsT[:, :].rearrange("(kc p) s -> p kc s", p=128)
                nc.sync.dma_start(out=T["wk_sb"], in_=wk_r)
                nc.sync.dma_start(out=T["hsT_sb"][:, :, 0:512], in_=hsT_r[:, :, 0:512])
                nc.sync.dma_start(out=T["wq_sb"], in_=wq_r)
                nc.sync.dma_start(out=T["wv_sb"], in_=wv_r)
                for sqc in range(1, SQC):
                    nc.sync.dma_start(
                        out=T["hsT_sb"][:, :, sqc * 512:(sqc + 1) * 512],
                        in_=hsT_r[:, :, sqc * 512:(sqc + 1) * 512],
                    )
                wo_r = wo[:, :].rearrange("(kc p) n -> p kc n", p=128)
                nc.sync.dma_start(out=T["wo_sb"], in_=wo_r)

            def emit_body(T, nextT, carried=False):
                hsT_sb = T["hsT_sb"]
                wq_sb = T["wq_sb"]
                wk_sb = T["wk_sb"]
                wv_sb = T["wv_sb"]
                wo_sb = T["wo_sb"]
                bq_sb = T["bq_sb"]
                bk_sb = T["bk_sb"]
                qT_sb = T["qT_sb"]
                kT_sb = T["kT_sb"]
                v_sb = T["v_sb"]
                ctxT_sb = T["ctxT_sb"]

                # ---- v ones column (per rep: v_sb rotates) ----
                nc.vector.tensor_copy(v_sb[:, :, :, 64:65], vones_f)

                # ---- helper emitters ----
                def qk_proj_pieces(w_sb, b_sb, dst, sqc, md, hs_sb=None, nsplit=2):
                    # one qT/kT projection block as nsplit self-contained
                    # drip items of KC/nsplit matmuls each. nsplit=4
                    # (~430ns PE per item) matches the per-chunk PE slack of
                    # the exp-bound steady state so a dripped projection
                    # never outruns the scores->exp rhythm; nsplit=2 keeps
                    # deadline-critical projections within their slot budget.
                    # Pieces accumulate through an fp32 SBUF partial (psum is
                    # never held across drip slots); the bias-add and merges
                    # run on DVE (ScalarE is exp-bound).
                    if hs_sb is None:
                        hs_sb = hsT_sb
                    ssl = slice(sqc * 512, (sqc + 1) * 512)
                    msl = slice(md * 128, (md + 1) * 128)
                    st = {}
                    per = KC // nsplit

                    def piece(i):
                        def run():
                            ps = pctx.tile([128, 512], mybir.dt.float32, tag="ctx", name="ps_qk")
                            for kc in range(per * i, per * (i + 1)):
                                nc.tensor.matmul(
                                    ps, w_sb[:, kc, msl], hs_sb[:, kc, ssl],
                                    start=(kc == per * i), stop=(kc == per * (i + 1) - 1),
                                )
                            if i == 0 and nsplit == 1:
                                nc.vector.tensor_scalar_add(dst[:, md, ssl], ps, b_sb[:, md:md + 1])
                            elif i == 0:
                                st["acc"] = qkp.tile([128, 512], F32, name="qk_part")
                                nc.vector.tensor_scalar_add(st["acc"], ps, b_sb[:, md:md + 1])
                            elif i < nsplit - 1:
                                acc2 = qkp.tile([128, 512], F32, name="qk_part")
                                nc.vector.tensor_add(acc2, ps, st["acc"])
                                st["acc"] = acc2
                            else:
                                nc.vector.tensor_add(dst[:, md, ssl], ps, st["acc"])
                        return run
                    return [piece(i) for i in range(nsplit)]

                def qk_proj(w_sb, b_sb, dst, sqc, md, hs_sb=None):
                    for p in qk_proj_pieces(w_sb, b_sb, dst, sqc, md, hs_sb, nsplit=2):
                        p()

                def v_proj(skc):
                    # psum from the ctx pool: during unit 0 (the only v_proj
                    # window) it has free slots, while taking a pbig buffer
                    # would single-buffer the scores psum under the exp stream.
                    ksl = slice(skc * 128, (skc + 1) * 128)
                    psv = pctx.tile([128, DG], mybir.dt.float32, tag="ctx", name="psv")
                    for kc in range(KC):
                        nc.tensor.matmul(
                            psv, hsT_sb[:, kc, ksl], wv_sb[:, kc, :],
                            start=(kc == 0), stop=(kc == KC - 1),
                        )
                    nc.vector.tensor_copy(
                        v_sb[:, skc, :, 0:64],
                        psv.rearrange("p (h d) -> p h d", h=HG),
                    )

                def epilogue(h, ctx, sqc):
                    # normalize ctxT_unnorm (rows 0:64) by l (row 64), write ctxT
                    poff = (h % 2) * 64
                    cpart = h // 2
                    ssl = slice(sqc * 512, (sqc + 1) * 512)
                    invl_r = ep.tile([65, 512], F32R, tag="invr", name="invl_r")
                    with nc.allow_low_precision(reason="1/l rounded to fp32r feeds the fp32r broadcast matmul"):
                        nc.vector.reciprocal(invl_r[64:65, :], ctx[64:65, :])
                    psb = pctx.tile([64, 512], mybir.dt.float32, tag="ctx", name="psb")
                    nc.tensor.matmul(
                        psb, ones_r[64:65, 0:64], invl_r[64:65, :],
                        start=True, stop=True,
                    )
                    bc = ep.tile([64, 512], mybir.dt.float32, tag="bc", name="bc")
                    nc.vector.tensor_copy(bc, psb)
                    nc.vector.tensor_mul(
                        ctxT_sb[poff:poff + 64, cpart, ssl],
                        ctx[0:64, :], bc,
                    )

                out_copy_toggle = [0]
                in_tail = [False]

                def out_proj(mo, sqc):
                    osl = slice(mo * 128, (mo + 1) * 128)
                    ssl = slice(sqc * 512, (sqc + 1) * 512)
                    pso = pctx.tile([128, 512], mybir.dt.float32, tag="ctx", name="pso")
                    for kc2 in range(2):
                        nc.tensor.matmul(
                            pso, wo_sb[:, kc2, osl], ctxT_sb[:, kc2, ssl],
                            start=(kc2 == 0), stop=(kc2 == 1),
                        )
                    ot = ost.tile([128, 512], BF16, name="ot")
                    if in_tail[0] and out_copy_toggle[0] % 2 == 0:
                        nc.scalar.copy(out=ot, in_=pso)
                    else:
                        nc.vector.tensor_copy(ot, pso)
                    out_copy_toggle[0] += 1
                    nc.sync.dma_start(out=outT[osl, ssl], in_=ot)

                # ---- pre-attention: ONLY kT(0,md0) + qT(0,md0) ----
                # These two projections alone gate unit (0,0)'s first scores,
                # so the exp stream (the ScalarE bottleneck) starts as soon as
                # wk+wq+hsT(sqc0) land. Every other projection and all v
                # chunks are deadline-ordered into the units' drip stream.
                if not carried:
                    qk_proj(wk_sb, bk_sb, kT_sb, 0, 0)
                    qk_proj(wq_sb, bq_sb, qT_sb, 0, 0)
                # when carried, these projections (plus kT(1,md0)) already ran
                # inside the previous rep's last-unit drip stream, on this
                # rep's tiles, so the first scores issue at the boundary.

                # deferred work, dripped into later attention units' loops.
                # prio_q (qT projections + ctx-psum-releasing epilogues) drains
                # ahead of slack_q and every iteration, so unit u's ctx slots free
                # within unit u+1 and qT(sqc) is ready one sqc ahead.
                from collections import deque
                epi_q = deque()    # epilogues: must land at slots 2-3 of the
                                   # following unit (they free the ctx psum
                                   # the unit's own PVs wait on)
                prio_q = deque()   # deadline-ordered projections
                slack_q = deque()  # out-projections

                def drip(slack_ok=True):
                    if epi_q:
                        epi_q.popleft()()
                    elif prio_q:
                        prio_q.popleft()()
                    elif slack_ok and slack_q:
                        slack_q.popleft()()

                # ---- attention units: (sqc, head-pair), software-pipelined ----
                # Each pair's two K=64 scoresT matmuls go to disjoint PE row
                # groups (partitions 0:64 / 64:128) and the two banks of one
                # [128, 1024] psum tile, so the PE overlaps them and a single exp
                # covers both heads.
                units = [(sqc, pair) for sqc in range(SQC) for pair in range(2)]
                for ui, (sqc, pair) in enumerate(units):
                    ssl = slice(sqc * 512, (sqc + 1) * 512)
                    ctxs = [None, None]
                    # deadline-ordered projection drips. kT(s, md) is read by
                    # every unit with pair==md at its skc=4s; qT(s, md) gates
                    # unit (s, md)'s start.
                    if ui == 0:
                        k_sqcs = (2, 3) if carried else (1, 2, 3)
                        for k_sqc in k_sqcs:
                            prio_q.extend(qk_proj_pieces(wk_sb, bk_sb, kT_sb, k_sqc, 0))
                        prio_q.extend(qk_proj_pieces(wk_sb, bk_sb, kT_sb, 0, 1))
                        prio_q.extend(qk_proj_pieces(wq_sb, bq_sb, qT_sb, 0, 1))
                        prio_q.extend(qk_proj_pieces(wk_sb, bk_sb, kT_sb, 1, 1))
                    if ui == 1:
                        for k_sqc in (2, 3):
                            prio_q.extend(qk_proj_pieces(wk_sb, bk_sb, kT_sb, k_sqc, 1, nsplit=4))
                        for md in range(2):
                            prio_q.extend(qk_proj_pieces(wq_sb, bq_sb, qT_sb, 1, md, nsplit=4))
                    if ui in (2, 3):
                        for md in range(2):
                            prio_q.extend(qk_proj_pieces(wq_sb, bq_sb, qT_sb, ui, md, nsplit=4))
                    # exp is written fp8 into [128, 2, 1024] chunk-pair tiles;
                    # PV runs as fp8 DoubleRow matmuls contracting 2 sk-chunks
                    # (K=256) per instruction at 2 MACs/cell/cycle.
                    prev = None
                    cur = None
                    for skc in range(SKC):
                        ksl = slice(skc * 128, (skc + 1) * 128)
                        t = skc % 2
                        if t == 0:
                            cur = expp.tile([128, 2, 1024], FP8, name="expT2")
                        sT = pbig.tile([128, 1024], mybir.dt.float32, tag="st", name="sT")
                        for hh in range(2):
                            nc.tensor.matmul(
                                sT[:, hh * 512:(hh + 1) * 512],
                                kT_sb[hh * 64:(hh + 1) * 64, pair, ksl],
                                qT_sb[hh * 64:(hh + 1) * 64, pair, ssl],
                                start=True, stop=True,
                            )
                        nc.scalar.activation(cur[:, t, :], sT, AFT.Exp, scale=SCALE)
                        if skc == 3:
                            # ctx psum allocated lazily, after the previous
                            # unit's first epilogue has dripped (skc==2): an
                            # eager alloc would fill the pool and deadlock the
                            # epilogue's own psum allocation.
                            ctxs[0] = pctx.tile([65, 512], mybir.dt.float32, tag="ctx", name="ctx0")
                            ctxs[1] = pctx.tile([65, 512], mybir.dt.float32, tag="ctx", name="ctx1")
                        if t == 1:
                            if prev is not None:
                                pexp, ppc = prev
                                for hh in range(2):
                                    nc.tensor.matmul(
                                        ctxs[hh],
                                        v_sb[:, 2 * ppc:2 * ppc + 2, pair * 2 + hh, 0:65],
                                        pexp[:, :, hh * 512:(hh + 1) * 512],
                                        start=(ppc == 0), stop=False,
                                        perf_mode=DR,
                                    )
                            prev = (cur, skc // 2)
                        if ui == 0:
                            v_proj(skc)  # stream all 16 v chunks under unit 0
                        if ui == 5 and skc == 0 and nextT is not None:
                            # prefetch the next rep's inputs now: they enter
                            # the sync queue ahead of this rep's tail output
                            # DMAs, the DMA engines are idle here, and the
                            # data is resident in time for the carried
                            # projections two units later.
                            emit_dmas(nextT)
                        if ui == 7 and skc == 0 and nextT is not None:
                            # carry the next rep's stream-gating projections
                            # (kT/qT sqc0 md0, kT sqc1 md0) into this unit's
                            # drip slots, so the next rep's first scores can
                            # issue right at the boundary.
                            prio_q.extend(qk_proj_pieces(
                                nextT["wk_sb"], nextT["bk_sb"], nextT["kT_sb"], 0, 0,
                                hs_sb=nextT["hsT_sb"]))
                            prio_q.extend(qk_proj_pieces(
                                nextT["wq_sb"], nextT["bq_sb"], nextT["qT_sb"], 0, 0,
                                hs_sb=nextT["hsT_sb"]))
                            prio_q.extend(qk_proj_pieces(
                                nextT["wk_sb"], nextT["bk_sb"], nextT["kT_sb"], 1, 0,
                                hs_sb=nextT["hsT_sb"]))
                        if skc >= 2:
                            drip()
                    pexp, ppc = prev
                    for hh in range(2):
                        nc.tensor.matmul(
                            ctxs[hh],
                            v_sb[:, 2 * ppc:2 * ppc + 2, pair * 2 + hh, 0:65],
                            pexp[:, :, hh * 512:(hh + 1) * 512],
                            start=(ppc == 0), stop=True,
                            perf_mode=DR,
                        )
                    # defer this unit's epilogues into the next unit's loop
                    for hh in range(2):
                        epi_q.append(
                            lambda h=pair * 2 + hh, ctx=ctxs[hh], sqc=sqc: epilogue(h, ctx, sqc))
                    if pair == 1:
                        # both pairs of this sqc done (after epilogues): queue out-proj
                        for mo in range(8):
                            slack_q.append(lambda mo=mo, sqc=sqc: out_proj(mo, sqc))

                # drain remaining deferred work (incl. any carried pieces for
                # the next rep that didn't fit this rep's drip slots)
                in_tail[0] = True
                while epi_q or prio_q or slack_q:
                    drip(slack_ok=True)
                in_tail[0] = False

            T = alloc_tiles()
            emit_dmas(T)
            for _rep in range(reps):
                nextT = alloc_tiles() if _rep + 1 < reps else None
                emit_body(T, nextT if carry else None, carried=(_rep > 0 and carry))
                if nextT is not None and not carry:
                    emit_dmas(nextT)
                T = nextT

    nc.compile()
    return nc


def _get_nc(reps=1):
    global _cached_nc
    if reps != 1:
        return _build_nc(reps)
    if _cached_nc is None:
        _cached_nc = _build_nc()
    return _cached_nc


def kernel(hidden_states, Wq, bq, Wk, bk, Wv, bv, Wo, bo, _want_trace=False):
    from concourse.bass_utils import run_bass_kernel_spmd

    hidden_states = np.asarray(hidden_states, dtype=np.float32)
    Wq = np.asarray(Wq, dtype=np.float32)
    Wk = np.asarray(Wk, dtype=np.float32)
    Wv = np.asarray(Wv, dtype=np.float32)
    Wo = np.asarray(Wo, dtype=np.float32)
    bq = np.asarray(bq, dtype=np.float32)
    bk = np.asarray(bk, dtype=np.float32)
    bv = np.asarray(bv, dtype=np.float32)
    bo = np.asarray(bo, dtype=np.float32)

    nc = _get_nc()

    import ml_dtypes
    bf16 = ml_dtypes.bfloat16
    hsTs = [np.ascontiguousarray(hidden_states[b].T).astype(bf16) for b in range(B)]
    in_maps = []
    for c in range(N_CORES):
        b, hg = divmod(c, HG)
        sl = slice(hg * DG, (hg + 1) * DG)
        in_maps.append({
            "hsT": hsTs[b],
            "wq": np.ascontiguousarray(Wq[:, sl]).astype(bf16),
            "wk": np.ascontiguousarray(Wk[:, sl]).astype(bf16),
            "wv": np.ascontiguousarray(Wv[:, sl]).astype(bf16),
            "wo": np.ascontiguousarray(Wo[sl, :]),
            "bq": np.ascontiguousarray(bq[sl].reshape(2, 128)),
            "bk": np.ascontiguousarray(bk[sl].reshape(2, 128)),
        })

    try:
        res = run_bass_kernel_spmd(
            nc, in_maps, core_ids=list(range(N_CORES)), trace=_want_trace,
        )
    except ModuleNotFoundError:
        res = run_bass_kernel_spmd(
            nc, in_maps, core_ids=list(range(N_CORES)), trace=False,
        )

    bias_full = bo + bv @ Wo  # [D]
    out = np.empty((B, S, D), dtype=np.float32)
    for b in range(B):
        acc = res.results[HG * b]["outT"].astype(np.float64)
        for g in range(1, HG):
            acc = acc + res.results[HG * b + g]["outT"]
        out[b] = acc.T + bias_full

    if _want_trace:
        return out, res
    return out

